# revision 1
# baseline (speedup 1.0000x reference)
"""Trainium2 Bass kernel v2 for the 5-layer GraphConv GNN.

Design (per core, 8 cores, dst-partitioned: core owns NPC=12500 dst nodes):
- 8 GPSIMD "sets" per core, one per src-chunk of 12500 nodes. Set s's 16
  partitions hold the gather table chunk s (yT rows, f32, d=1) - the full
  table lives across the 128 partitions once, no replication.
- Edges grouped (set = src chunk, section = dst-node range of NS nodes),
  sorted by dst within a section, one leading dummy slot per section.
- Per section: ap_gather slots (Pool) -> in-place mult by edge weight (DVE)
  -> tensor_tensor_scan cumsum (DVE) -> ap_gather node endpoints (Pool)
  -> diff (DVE) into per-set partials P [128, ~NPC] bf16.
- Cross-set reduction + W application fused into PE matmuls:
  z = SEL^T @ P + W_root^T @ xT (PSUM accumulate), ACT relu+bias -> xT_next.
- Layer tables: L0 gathers raw x (6 dims); later layers gather
  y_l = x_l @ W_rel_l (dims 15,10,5,2), built on device + AllGather.
- Final layer: softmax over 2 = sigmoid of logit diff (saturates exactly).
"""

import sys
sys.path.insert(0, '/opt/trn_rl_repo')
import numpy as np
import ml_dtypes

N_NODES = 100000
N_CORES = 8
NPC = N_NODES // N_CORES          # 12500 dst nodes per core
NSETS = 8
CHUNK = N_NODES // NSETS          # 12500 src nodes per set-chunk
DIMS = [6, 20, 15, 10, 5, 2]
L = 5
GD = [6, 15, 10, 5, 2]            # gathered dims per layer
NSEC = 32
NS = (NPC + NSEC - 1) // NSEC     # nodes per section
NEPI = ((NS + 1 + 31) // 32) * 32  # endpoint idxs per section (32-aligned: Q7 reads idxs as 32-bit words, so per-section idx slices must stay 4B-aligned)
P_W = (NSEC - 1) * NS + NEPI - 1 + 1  # partials width (12560)

bf = ml_dtypes.bfloat16


def preprocess(edge_index, edge_weight):
    """Build per-core idx/w/endpoint arrays. Returns (idx16, eidx16, wtab, seclen)."""
    src = np.asarray(edge_index[0], dtype=np.int64)
    dst = np.asarray(edge_index[1], dtype=np.int64)
    w = np.asarray(edge_weight, dtype=np.float32)

    core = dst // NPC
    nloc = dst % NPC
    st = src // CHUNK
    sloc = (src - st * CHUNK).astype(np.int64)
    sec = nloc // NS
    nsec = nloc - sec * NS
    group = ((core * NSETS + st) * NSEC + sec).astype(np.int64)
    ngroups = N_CORES * NSETS * NSEC

    okey = group * NPC + nloc
    order = np.argsort(okey, kind='stable')
    g_s, sloc_s, w_s = group[order], sloc[order], w[order]

    counts = np.bincount(g_s, minlength=ngroups)
    seclen = int(counts.max()) + 1            # +1 dummy slot at pos 0
    seclen = ((seclen + 31) // 32) * 32       # 32-aligned (idx slice alignment)

    # slot positions within each group (dummy at 0)
    starts = np.concatenate([[0], np.cumsum(counts)[:-1]])
    pos = (np.arange(len(g_s)) - starts[g_s]) + 1

    idx_p = np.zeros((ngroups, seclen), np.int16)
    w_p = np.zeros((ngroups, seclen), np.float32)
    idx_p[g_s, pos] = sloc_s.astype(np.int16)
    w_p[g_s, pos] = w_s

    # endpoints: C[n] = 1-based position of last slot of node n (dummy-incl)
    gn_key = group * NS + nsec
    cnt_gn = np.bincount(gn_key, minlength=ngroups * NS).reshape(ngroups, NS)
    C = np.cumsum(cnt_gn, axis=1)             # [ngroups, NS]
    ep = np.zeros((ngroups, NEPI), np.int16)
    ep[:, 1:NS + 1] = C.astype(np.int16)
    if NS + 1 < NEPI:
        ep[:, NS + 1:] = C[:, -1:].astype(np.int16)

    # wrap into SBUF layouts per core
    def wrap16(a):
        # [NSETS, NSEC, M] -> [128, NSEC*M/16]: item j of (set s, sec) at
        # [16*s + j%16, sec*(M//16) + j//16]
        ns, nsec_, m = a.shape
        aw = a.reshape(ns, nsec_, m // 16, 16).transpose(0, 3, 1, 2)
        return np.ascontiguousarray(aw.reshape(ns * 16, nsec_ * (m // 16)))

    idx16 = np.zeros((N_CORES, 128, NSEC * seclen // 16), np.int16)
    eidx16 = np.zeros((N_CORES, 128, NSEC * NEPI // 16), np.int16)
    wtab = np.zeros((N_CORES, 128, NSEC * seclen), bf)
    for c in range(N_CORES):
        blk = slice(c * NSETS * NSEC, (c + 1) * NSETS * NSEC)
        idx_c = idx_p[blk].reshape(NSETS, NSEC, seclen)
        ep_c = ep[blk].reshape(NSETS, NSEC, NEPI)
        w_c = w_p[blk].reshape(NSETS, NSEC, seclen)
        idx16[c] = wrap16(idx_c)
        eidx16[c] = wrap16(ep_c)
        # w duplicated across the 16 partitions of each set, sections concat
        wtab[c] = np.repeat(
            w_c.reshape(NSETS, 1, NSEC * seclen), 16, axis=1
        ).reshape(128, NSEC * seclen).astype(bf)
    return idx16, eidx16, wtab, seclen


def build_gnn(nc, seclen, debug=False, reps=1):
    import concourse.tile as tile
    from concourse import mybir
    f32 = mybir.dt.float32
    bf16 = mybir.dt.bfloat16
    i16 = mybir.dt.int16
    AF = mybir.ActivationFunctionType
    OP = mybir.AluOpType

    SL = seclen
    # ---- DRAM I/O ----
    xtab_d = nc.dram_tensor("xtab", [128, CHUNK], f32, kind="ExternalInput")
    xT0_d = nc.dram_tensor("xT0", [DIMS[0], NPC], bf16, kind="ExternalInput")
    idx_d = nc.dram_tensor("idx16", [128, NSEC * SL // 16], i16, kind="ExternalInput")
    eidx_d = nc.dram_tensor("eidx16", [128, NSEC * NEPI // 16], i16, kind="ExternalInput")
    wtab_d = nc.dram_tensor("wtab", [128, NSEC * SL], bf16, kind="ExternalInput")
    sel_d = nc.dram_tensor("sel", [128, 16], bf16, kind="ExternalInput")
    dv_d = nc.dram_tensor("dv", [2, 1], bf16, kind="ExternalInput")
    selw0_d = nc.dram_tensor("selw0", [128, DIMS[1]], bf16, kind="ExternalInput")
    wrel_d = [nc.dram_tensor(f"wrel{l}", [DIMS[l], DIMS[l + 1]], bf16, kind="ExternalInput")
              for l in range(1, L)]
    wroot_d = [nc.dram_tensor(f"wroot{l}", [DIMS[l], DIMS[l + 1]], bf16, kind="ExternalInput")
               for l in range(L)]
    brel_d = [nc.dram_tensor(f"brel{l}", [DIMS[l + 1], 1], f32, kind="ExternalInput")
              for l in range(L)]
    out_d = nc.dram_tensor("out", [NPC, DIMS[L]], f32, kind="ExternalOutput")
    dbg = {}
    if debug:
        for l in range(L):
            dbg[f"P{l}"] = nc.dram_tensor(f"dbgP{l}", [128, P_W], bf16, kind="ExternalOutput")
            if l < L - 1:
                dbg[f"xT{l + 1}"] = nc.dram_tensor(
                    f"dbgxT{l + 1}", [DIMS[l + 1], NPC], bf16, kind="ExternalOutput")
        for sec in range(2):
            for nm in ("g", "m", "S"):
                dbg[f"{nm}{sec}"] = nc.dram_tensor(
                    f"dbg{nm}{sec}", [128, SL], f32, kind="ExternalOutput")
            dbg[f"E{sec}"] = nc.dram_tensor(
                f"dbgE{sec}", [128, NEPI], f32, kind="ExternalOutput")

    groups = [list(range(N_CORES))]
    NT = (NPC + 511) // 512        # z-assembly column tiles

    with tile.TileContext(nc) as tc:
        with (
            tc.tile_pool(name="const", bufs=1) as cpool,
            tc.tile_pool(name="table", bufs=1) as tpool,
            tc.tile_pool(name="xt", bufs=1) as xpool,
            tc.tile_pool(name="sect", bufs=2) as spool,
            tc.tile_pool(name="pp", bufs=1) as ppool_sb,
            tc.tile_pool(name="zps", bufs=2, space="PSUM") as zpool,
            tc.tile_pool(name="ysmall", bufs=3) as ypool,
            tc.tile_pool(name="dram", bufs=1, space="DRAM") as dpool,
        ):
            # ---- constants ----
            idx_sb = cpool.tile([128, NSEC * SL // 16], i16)
            nc.sync.dma_start(idx_sb[:], idx_d.ap()[:])
            eidx_sb = cpool.tile([128, NSEC * NEPI // 16], i16)
            nc.sync.dma_start(eidx_sb[:], eidx_d.ap()[:])
            sel_sb = cpool.tile([128, 16], bf16)
            nc.sync.dma_start(sel_sb[:], sel_d.ap()[:])
            dv_sb = cpool.tile([2, 1], bf16)
            nc.sync.dma_start(dv_sb[:], dv_d.ap()[:])
            selw0_sb = cpool.tile([128, DIMS[1]], bf16)
            nc.sync.dma_start(selw0_sb[:], selw0_d.ap()[:])
            wrel_sb = {}
            for i, l in enumerate(range(1, L)):
                t = cpool.tile([DIMS[l], DIMS[l + 1]], bf16, name=f"wrel_sb{l}")
                nc.sync.dma_start(t[:], wrel_d[i].ap()[:])
                wrel_sb[l] = t
            wroot_sb, brel_sb = [], []
            for l in range(L):
                t = cpool.tile([DIMS[l], DIMS[l + 1]], bf16, name=f"wroot_sb{l}")
                nc.sync.dma_start(t[:], wroot_d[l].ap()[:])
                wroot_sb.append(t)
                t2 = cpool.tile([DIMS[l + 1], 1], f32, name=f"brel_sb{l}")
                nc.sync.dma_start(t2[:], brel_d[l].ap()[:])
                brel_sb.append(t2)

            def emit_body(rep):
              xT_cur = xpool.tile([DIMS[0], NPC], bf16, name=f"xT0_{rep}", tag="xT", bufs=2)
              nc.sync.dma_start(xT_cur[:], xT0_d.ap()[:])

              for l in range(L):
                cg = GD[l]
                din = DIMS[l]
                cout = DIMS[l + 1]
                # ---- gather table ----
                table_sb = tpool.tile([128, CHUNK], f32, name=f"table{rep}_{l}", tag="tab")
                if l == 0:
                    nc.sync.dma_start(table_sb[:], xtab_d.ap()[:])
                else:
                    ybounce = dpool.tile([cg, NPC], f32, name=f"ybounce{rep}_{l}")
                    for t in range(NT):
                        c0, c1 = t * 512, min((t + 1) * 512, NPC)
                        yps = zpool.tile([cg, 512], f32, name=f"yps{rep}_{l}_{t}", tag="yps")
                        nc.tensor.matmul(
                            out=yps[:, :c1 - c0],
                            lhsT=wrel_sb[l][:],
                            rhs=xT_cur[:, c0:c1],
                            start=True, stop=True,
                        )
                        ysb = ypool.tile([cg, 512], f32, name=f"ysb{rep}_{l}_{t}", tag="ysb")
                        nc.scalar.activation(
                            out=ysb[:, :c1 - c0], in_=yps[:, :c1 - c0],
                            func=AF.Copy)
                        nc.sync.dma_start(ybounce[:, c0:c1], ysb[:, :c1 - c0])
                    ytab = dpool.tile([NSETS * cg, NPC], f32, name=f"ytab{rep}_{l}")
                    nc.gpsimd.collective_compute(
                        "AllGather", mybir.AluOpType.bypass,
                        replica_groups=groups,
                        ins=[ybounce[:].opt()],
                        outs=[ytab[:].opt()],
                    )
                    for s in range(NSETS):
                        nc.sync.dma_start(
                            table_sb[16 * s:16 * s + cg, :],
                            ytab[s * cg:(s + 1) * cg, :])

                # ---- edge grind ----
                P = ppool_sb.tile([128, P_W], bf16, name=f"P{rep}_{l}", tag="P")
                for sec in range(NSEC):
                    w_t = spool.tile([128, SL], bf16, name=f"w{rep}_{l}_{sec}", tag="w")
                    nc.sync.dma_start(w_t[:], wtab_d.ap()[:, sec * SL:(sec + 1) * SL])
                    gth = spool.tile([128, SL], f32, name=f"g{rep}_{l}_{sec}", tag="g")
                    nc.gpsimd.ap_gather(
                        out_ap=gth[:], in_ap=table_sb[:],
                        idxs_ap=idx_sb[:, sec * (SL // 16):(sec + 1) * (SL // 16)],
                        channels=128, num_elems=CHUNK, d=1, num_idxs=SL,
                    )
                    msg = spool.tile([128, SL], f32, name=f"m{rep}_{l}_{sec}", tag="m")
                    nc.vector.tensor_tensor(
                        out=msg[:], in0=gth[:], in1=w_t[:], op=OP.mult)
                    S = spool.tile([128, SL], f32, name=f"S{rep}_{l}_{sec}", tag="S")
                    nc.vector.tensor_tensor_scan(
                        out=S[:], data0=msg[:], data1=msg[:], initial=0.0,
                        op0=OP.add, op1=OP.bypass)
                    E = spool.tile([128, NEPI], f32, name=f"E{rep}_{l}_{sec}", tag="E")
                    nc.gpsimd.ap_gather(
                        out_ap=E[:], in_ap=S[:],
                        idxs_ap=eidx_sb[:, sec * (NEPI // 16):(sec + 1) * (NEPI // 16)],
                        channels=128, num_elems=SL, d=1, num_idxs=NEPI,
                    )
                    nc.vector.tensor_tensor(
                        out=P[:, sec * NS:sec * NS + NEPI - 1],
                        in0=E[:, 1:NEPI], in1=E[:, 0:NEPI - 1], op=OP.subtract)
                    if debug and l == 0 and sec < 2:
                        nc.sync.dma_start(dbg[f"g{sec}"].ap()[:], gth[:])
                        nc.sync.dma_start(dbg[f"m{sec}"].ap()[:], msg[:])
                        nc.sync.dma_start(dbg[f"S{sec}"].ap()[:], S[:])
                        nc.sync.dma_start(dbg[f"E{sec}"].ap()[:], E[:])
                if debug:
                    nc.sync.dma_start(dbg[f"P{l}"].ap()[:], P[:])

                # ---- z assembly (per 512-column tile) ----
                if l < L - 1:
                    xT_next = xpool.tile([cout, NPC], bf16, name=f"xT{rep}_{l + 1}",
                                         tag="xT", bufs=2)
                else:
                    xT_next = None
                for t in range(NT):
                    c0, c1 = t * 512, min((t + 1) * 512, NPC)
                    cw = c1 - c0
                    zps = zpool.tile([cout, 512], f32, name=f"z{rep}_{l}_{t}", tag="zps")
                    nc.tensor.matmul(
                        out=zps[:, :cw],
                        lhsT=(selw0_sb if l == 0 else sel_sb[:, :cg])[:],
                        rhs=P[:, c0:c1], start=True, stop=False)
                    nc.tensor.matmul(
                        out=zps[:, :cw], lhsT=wroot_sb[l][:],
                        rhs=xT_cur[:, c0:c1], start=False, stop=True)
                    if l < L - 1:
                        nc.scalar.activation(
                            out=xT_next[:, c0:c1], in_=zps[:, :cw],
                            func=AF.Relu, bias=brel_sb[l][:])
                    else:
                        # relu, then softmax over 2 = sigmoid(z0 - z1)
                        zrt = ypool.tile([2, 512], bf16, name=f"zrt{rep}_{t}", tag="zrt")
                        nc.scalar.activation(
                            out=zrt[:, :cw], in_=zps[:, :cw],
                            func=AF.Relu, bias=brel_sb[l][:])
                        dps = zpool.tile([1, 512], f32, name=f"dps{rep}_{t}", tag="dps")
                        nc.tensor.matmul(
                            out=dps[:, :cw], lhsT=dv_sb[:],
                            rhs=zrt[:, :cw], start=True, stop=True)
                        p0 = ypool.tile([1, 512], f32, name=f"p0_{rep}_{t}", tag="p0")
                        nc.scalar.activation(out=p0[:, :cw], in_=dps[:, :cw],
                                             func=AF.Sigmoid)
                        p1 = ypool.tile([1, 512], f32, name=f"p1_{rep}_{t}", tag="p1")
                        nc.scalar.activation(out=p1[:, :cw], in_=dps[:, :cw],
                                             func=AF.Sigmoid, scale=-1.0)
                        nc.sync.dma_start(
                            out_d.ap()[c0:c1, 0:1].rearrange("n o -> o n"), p0[:, :cw])
                        nc.sync.dma_start(
                            out_d.ap()[c0:c1, 1:2].rearrange("n o -> o n"), p1[:, :cw])
                if debug and l < L - 1:
                    nc.sync.dma_start(dbg[f"xT{l + 1}"].ap()[:], xT_next[:])
                xT_cur = xT_next

            for rep in range(reps):
                emit_body(rep)
    return dbg


def make_host_inputs(inputs):
    x = np.asarray(inputs["x"], np.float32)
    idx16, eidx16, wtab, seclen = preprocess(
        inputs["edge_index"], inputs["edge_weight"])
    xtab = np.zeros((128, CHUNK), np.float32)
    for s in range(NSETS):
        xtab[16 * s:16 * s + 6, :] = x[s * CHUNK:(s + 1) * CHUNK, :].T
    sel = np.zeros((128, 16), bf)
    for p in range(128):
        sel[p, p % 16] = 1
    dv = np.array([[1.0], [-1.0]], np.float32).astype(bf)
    wrel0 = np.asarray(inputs["w_rel0"], np.float32)   # [6, 20]
    wrel0_pad = np.zeros((16, DIMS[1]), np.float32)
    wrel0_pad[:6] = wrel0
    selw0 = (sel.astype(np.float32) @ wrel0_pad).astype(bf)

    common = {"xtab": xtab, "sel": sel, "selw0": selw0, "dv": dv}
    for l in range(1, L):
        common[f"wrel{l}"] = np.asarray(inputs[f"w_rel{l}"], np.float32).astype(bf)
    for l in range(L):
        common[f"wroot{l}"] = np.asarray(inputs[f"w_root{l}"], np.float32).astype(bf)
        common[f"brel{l}"] = np.asarray(inputs[f"b_rel{l}"], np.float32).reshape(-1, 1)
    in_maps = []
    for c in range(N_CORES):
        m = dict(common)
        m["idx16"] = idx16[c]
        m["eidx16"] = eidx16[c]
        m["wtab"] = wtab[c]
        m["xT0"] = np.ascontiguousarray(
            x[c * NPC:(c + 1) * NPC, :].T).astype(bf)
        in_maps.append(m)
    return in_maps, seclen


def _install_loud_hook():
    import traceback
    from concourse import bass2jax
    bass2jax.install_neuronx_cc_hook()
    try:
        import libneuronxla
    except ImportError:
        return
    hook = libneuronxla.neuronx_cc
    def loud(*a, **k):
        try:
            return hook(*a, **k)
        except BaseException:
            traceback.print_exc()
            raise
    libneuronxla.neuronx_cc = loud
    bass2jax.install_neuronx_cc_hook = lambda: None


def run_gnn(inputs, trace=False, debug=False, reps=1):
    import concourse.bacc as bacc
    from concourse.bass_utils import run_bass_kernel_spmd
    _install_loud_hook()
    in_maps, seclen = make_host_inputs(inputs)
    nc = bacc.Bacc("TRN2", target_bir_lowering=False, debug=False,
                   num_devices=N_CORES)
    build_gnn(nc, seclen, debug=debug, reps=reps)
    nc.compile()
    res = run_bass_kernel_spmd(nc, in_maps, core_ids=list(range(N_CORES)),
                               trace=trace)
    out = np.concatenate([res.results[k]["out"] for k in range(N_CORES)], axis=0)
    return out, res


def kernel(**inputs):
    out, _ = run_gnn(inputs)
    return out


def kernel_traced(**inputs):
    """Returns (out, BassKernelResults). exec_time_ns is None when NTFF
    profiling is unavailable (axon client without the hook)."""
    return run_gnn(inputs, trace=True)



# revision 6
# speedup vs baseline: 1.0597x; 1.0597x over previous
"""Trainium2 Bass kernel v3 for the 5-layer GraphConv GNN.

Design (8 cores, dst-partitioned: core owns NPC=12500 dst nodes):
- Per layer l, the gather table is y_l = x_l @ W_rel_l for ALL nodes
  (node-major rows, 256B pitch in DRAM), built on device per layer:
  per-core slice via PE matmul + PE transpose, DMA to DRAM, AllGather.
  Aggregating y instead of x makes the edge aggregation directly produce
  z_rel (linearity of segment-sum).
- Edge phase via SWDGE dma_gather (4 queues): token-per-partition layout.
  Edges sorted by (window of 64 dst, src-quarter); each (win, q) list is
  padded to whole 128-edge chunks (chunk count = max over cores so the
  SPMD program is identical). Each chunk -> one PE matmul:
      zps[64 dst, cg] += onehot[128 edge, 64 dst]^T @ msgw[128 edge, cg]
  with host-precomputed fp8 0/1 one-hot (streamed from DRAM) and
  per-edge weights applied by one DVE broadcast-multiply per (group, q).
- Root term fused into the same PSUM: zps += xT[:, win]^T @ W_root.
- PE transpose -> [cout, 64] -> ACT relu+bias -> xT_next (feature-major).
- Final layer: softmax over 2 = sigmoid(+/-(z0 - z1)) via a [2,2] +/-1
  matmul and one dual-partition sigmoid.
- src index int16 limit (32767) -> 4 src "quarters" of 25088 table rows.
"""

import sys
sys.path.insert(0, '/opt/trn_rl_repo')
import numpy as np
import ml_dtypes

N_NODES = 100000
N_CORES = 8
NPC = N_NODES // N_CORES          # 12500 dst nodes per core
NPCP = 12544                      # padded (196 windows * 64)
W = 64                            # dst window size
NWIN = NPCP // W                  # 196
WPG = 14                          # windows per group
NGRP = NWIN // WPG                # 14
NGI = NPCP // 128                 # 98 node groups of 128 (for table build)
GIPG = NGI // NGRP                # 7 per group
NQ = 4                            # src quarters
QROWS = 25088                     # table rows per quarter
TROWS = N_CORES * NPCP            # 100352 == NQ * QROWS
assert TROWS == NQ * QROWS
DIMS = [6, 20, 15, 10, 5, 2]
L = 5
GD = [20, 15, 10, 5, 2]           # gathered dims per layer (= DIMS[l+1])
EL = [20, 16, 10, 6, 2]           # gather payload (bf16 elems, 4B-aligned)
ELMAX = EL[0]
PITCH = 128                       # table row pitch in bf16 elems (256B)

bf = ml_dtypes.bfloat16
f8 = ml_dtypes.float8_e4m3


def preprocess(edge_index, edge_weight):
    """Chunk layout + per-core idx/onehot/weight arrays."""
    src = np.asarray(edge_index[0], dtype=np.int64)
    dst = np.asarray(edge_index[1], dtype=np.int64)
    wgt = np.asarray(edge_weight, dtype=np.float32)

    row = (src // NPC) * NPCP + (src % NPC)       # ytab row of src
    q = row // QROWS
    lrow = (row % QROWS).astype(np.int16)
    core = dst // NPC
    d = dst % NPC
    win = d // W
    colw = (d % W).astype(np.int64)

    # counts per (core, win, q) -> shared chunk capacity
    bucket = (core * NWIN + win) * NQ + q
    cnt = np.bincount(bucket, minlength=N_CORES * NWIN * NQ).reshape(
        N_CORES, NWIN, NQ)
    NCH = np.maximum(-(-cnt.max(axis=0) // 128), 1)   # [NWIN, NQ]

    # chunk ordering: group-major, then q, then win within group
    cid0 = np.zeros((NWIN, NQ), np.int64)
    gs = np.zeros((NGRP, NQ), np.int64)
    gc = np.zeros((NGRP, NQ), np.int64)
    tot = 0
    for g in range(NGRP):
        for qi in range(NQ):
            gs[g, qi] = tot
            for wi in range(g * WPG, (g + 1) * WPG):
                cid0[wi, qi] = tot
                tot += NCH[wi, qi]
            gc[g, qi] = tot - gs[g, qi]
    TOTCH = tot
    MAXGC = int(gc.max())

    # per-edge rank within its (core, win, q) bucket
    order = np.argsort(bucket, kind='stable')
    inv = np.empty_like(order)
    inv[order] = np.arange(len(order))
    flat_cnt = cnt.reshape(-1)
    starts = np.concatenate([[0], np.cumsum(flat_cnt)[:-1]])
    rank = inv - starts[bucket]
    slot = cid0[win, q] * 128 + rank              # global slot id (per core)

    percore = []
    for c in range(N_CORES):
        m = core == c
        s = slot[m]
        idxw16 = np.zeros((16, TOTCH * 8), np.int16)
        idxw16[s % 16, s // 16] = lrow[m]
        oh = np.zeros((128, TOTCH * 64), f8)
        oh[s % 128, (s // 128) * 64 + colw[m]] = 1.0
        wts = np.zeros((128, TOTCH), bf)
        wts[s % 128, s // 128] = wgt[m]
        percore.append({
            "idxw": np.tile(idxw16, (8, 1)),
            "oh": oh,
            "wts": wts,
        })

    plan = {"NCH": NCH, "cid0": cid0, "gs": gs, "gc": gc,
            "TOTCH": TOTCH, "MAXGC": MAXGC}
    return plan, percore


def _dma_gather_small(g, out_ap, in_ap, idxs_ap, num_idxs, elem_size,
                      elem_step, queue_num):
    """bass dma_gather (HBM, non-transpose) minus the 256B elem assert.
    Row PITCH (elem_step) must still be a 256B multiple."""
    from concourse import mybir, ap_utils
    from concourse.bass import MemorySpace
    from concourse._compat import exact_div, round_up_to_multiple
    g._assert_queue_num(queue_num)
    assert idxs_ap.dtype == mybir.dt.int16
    assert in_ap.dtype == out_ap.dtype
    assert (elem_size * mybir.dt.size(in_ap.dtype)) % 4 == 0
    assert in_ap.space == MemorySpace.DRAM
    assert ap_utils.ap_is_contiguous(out_ap.ap[1:])
    assert ap_utils.ap_is_contiguous(idxs_ap.ap[1:])
    assert in_ap.ap[-1][1] == elem_size
    assert out_ap.ap[-1][1] == elem_size
    assert out_ap.ap[0][1] * out_ap.ap[1][1] == round_up_to_multiple(num_idxs, 128)
    assert in_ap.ap[0][0] == elem_step
    stride_bytes_256 = exact_div(elem_step * mybir.dt.size(in_ap.dtype), 256)
    assert 0 < stride_bytes_256 < 256
    _in_ap = g.lower_ap_dma(in_ap, for_custom_bir_dma=True)
    return g.add_instruction(
        mybir.InstDMAGatherAnt(
            name=g.bass.get_next_instruction_name(),
            ins=[*_in_ap, g.lower_ap(idxs_ap),
                 g.lower_val_access(g.to_reg(num_idxs))],
            outs=[g.lower_ap(out_ap)],
            transpose=False,
            num_idxs=num_idxs,
            elem_size=elem_size,
            stride_bytes_256=stride_bytes_256,
            gen_mode=0,
            single_packet=False,
            queue_num=queue_num,
            sbuf_tokens_per_rank=0,
            sbuf_free_dim_per_rank=0,
            sbuf_free_dim_pad_per_rank=0,
            sbuf_byte_offset=0,
        ))


def build_gnn(nc, plan, reps=1):
    import concourse.tile as tile
    from concourse import mybir
    f32 = mybir.dt.float32
    bf16 = mybir.dt.bfloat16
    i16 = mybir.dt.int16
    fp8 = mybir.dt.float8e4
    AF = mybir.ActivationFunctionType
    OP = mybir.AluOpType

    NCH, cid0, gs, gc = plan["NCH"], plan["cid0"], plan["gs"], plan["gc"]
    TOTCH, MAXGC = plan["TOTCH"], plan["MAXGC"]
    groups = [list(range(N_CORES))]

    # ---- DRAM I/O ----
    idx_d = nc.dram_tensor("idxw", [128, TOTCH * 8], i16, kind="ExternalInput")
    oh_d = nc.dram_tensor("oh", [128, TOTCH * 64], fp8, kind="ExternalInput")
    wts_d = nc.dram_tensor("wts", [128, TOTCH], bf16, kind="ExternalInput")
    xT0_d = nc.dram_tensor("xT0", [DIMS[0], NPCP], bf16, kind="ExternalInput")
    ident_d = nc.dram_tensor("ident", [64, 64], bf16, kind="ExternalInput")
    dv_d = nc.dram_tensor("dv", [2, 2], bf16, kind="ExternalInput")
    wrel_d = [nc.dram_tensor(f"wrel{l}", [DIMS[l], GD[l]], bf16,
                             kind="ExternalInput") for l in range(L)]
    wroot_d = [nc.dram_tensor(f"wroot{l}", [DIMS[l], DIMS[l + 1]], bf16,
                              kind="ExternalInput") for l in range(L)]
    brel_d = [nc.dram_tensor(f"brel{l}", [DIMS[l + 1], 1], f32,
                             kind="ExternalInput") for l in range(L)]
    out_d = nc.dram_tensor("out", [2, NPC], f32, kind="ExternalOutput")

    with tile.TileContext(nc) as tc:
        with (
            tc.tile_pool(name="const", bufs=1) as cpool,
            tc.tile_pool(name="xt", bufs=1) as xpool,
            tc.tile_pool(name="stream", bufs=2) as spool,
            tc.tile_pool(name="msgp", bufs=2) as mpool,
            tc.tile_pool(name="small", bufs=3) as ypool,
            tc.tile_pool(name="zp", bufs=3, space="PSUM") as zpool,
            tc.tile_pool(name="tp", bufs=3, space="PSUM") as tpool,
            tc.tile_pool(name="dram", bufs=1, space="DRAM") as dpool,
        ):
            # ---- constants ----
            wts_sb = cpool.tile([128, TOTCH], bf16)
            nc.sync.dma_start(wts_sb[:], wts_d.ap()[:])
            ident_sb = cpool.tile([64, 64], bf16)
            nc.sync.dma_start(ident_sb[:], ident_d.ap()[:])
            dv_sb = cpool.tile([2, 2], bf16)
            nc.sync.dma_start(dv_sb[:], dv_d.ap()[:])
            wrel_sb, wroot_sb, brel_sb = [], [], []
            for l in range(L):
                t = cpool.tile([DIMS[l], GD[l]], bf16, name=f"wrel_sb{l}")
                nc.sync.dma_start(t[:], wrel_d[l].ap()[:])
                wrel_sb.append(t)
                t = cpool.tile([DIMS[l], DIMS[l + 1]], bf16, name=f"wroot_sb{l}")
                nc.sync.dma_start(t[:], wroot_d[l].ap()[:])
                wroot_sb.append(t)
                t = cpool.tile([DIMS[l + 1], 1], f32, name=f"brel_sb{l}")
                nc.sync.dma_start(t[:], brel_d[l].ap()[:])
                brel_sb.append(t)
            pout = cpool.tile([2, NPCP], f32, name="pout")

            def emit_body(rep):
                ytab = [dpool.tile([TROWS, PITCH], bf16, name=f"ytab{rep}_{i}")
                        for i in range(2)]
                ybounce = dpool.tile([NPCP, PITCH], bf16, name=f"yb{rep}")

                def build_table(xT, l, gi_lo, gi_hi, ybsb):
                    """y_l = x_l @ W_rel_l for node groups [gi_lo, gi_hi)."""
                    cg = GD[l]
                    for gi in range(gi_lo, gi_hi):
                        yps = zpool.tile([GD[0], 128], f32,
                                         name=f"yps{rep}_{l}_{gi}", tag="yps",
                                         bufs=2)
                        nc.tensor.matmul(
                            out=yps[0:cg, :], lhsT=wrel_sb[l][:],
                            rhs=xT[:, gi * 128:(gi + 1) * 128],
                            start=True, stop=True)
                        ysb = ypool.tile([GD[0], 128], bf16,
                                         name=f"ysb{rep}_{l}_{gi}", tag="ysb",
                                         bufs=2)
                        nc.scalar.activation(out=ysb[0:cg, :], in_=yps[0:cg, :],
                                             func=AF.Copy)
                        ytp = tpool.tile([128, GD[0]], bf16,
                                         name=f"ytp{rep}_{l}_{gi}", tag="ytp",
                                         bufs=1)
                        nc.tensor.transpose(ytp[:, 0:cg], ysb[0:cg, :],
                                            ident_sb[0:cg, 0:cg])
                        nc.scalar.activation(out=ybsb[:, gi, 0:cg],
                                             in_=ytp[:, 0:cg], func=AF.Copy)

                def flush_table(l, ybsb):
                    cg = GD[l]
                    nc.sync.dma_start(
                        ybounce[:, 0:cg].rearrange("(g p) e -> p g e", p=128),
                        ybsb[:, :, 0:cg])
                    nc.gpsimd.collective_compute(
                        "AllGather", mybir.AluOpType.bypass,
                        replica_groups=groups,
                        ins=[ybounce[:].opt()],
                        outs=[ytab[l % 2][:].opt()],
                    )

                # initial table (layer 0) from xT0
                xT_cur = xpool.tile([DIMS[0], NPCP], bf16,
                                    name=f"xT0_{rep}", tag="xT", bufs=2)
                nc.sync.dma_start(xT_cur[:], xT0_d.ap()[:])
                ybsb0 = ypool.tile([128, NGI, GD[0]], bf16,
                                   name=f"ybsb{rep}_0", tag="ybsb", bufs=2)
                build_table(xT_cur, 0, 0, NGI, ybsb0)
                flush_table(0, ybsb0)

                for l in range(L):
                    cg = GD[l]          # == DIMS[l + 1] == cout
                    elem = EL[l]
                    ytab_cur = ytab[l % 2]
                    if l < L - 1:
                        xT_next = xpool.tile([cg, NPCP], bf16,
                                             name=f"xT{rep}_{l + 1}", tag="xT",
                                             bufs=2)
                        ybsb = ypool.tile([128, NGI, GD[0]], bf16,
                                          name=f"ybsb{rep}_{l + 1}", tag="ybsb",
                                          bufs=2)
                    for g in range(NGRP):
                        g0 = int(gs[g, 0])
                        totg = int(gc[g].sum())
                        idx_sb = spool.tile([128, MAXGC * NQ * 8], i16,
                                            name=f"ix{rep}_{l}_{g}", tag="ix",
                                            bufs=2)
                        nc.sync.dma_start(
                            idx_sb[:, 0:totg * 8],
                            idx_d.ap()[:, g0 * 8:(g0 + totg) * 8])
                        oh_sb = spool.tile([128, MAXGC * NQ * 64], fp8,
                                           name=f"oh{rep}_{l}_{g}", tag="oh",
                                           bufs=2)
                        nc.sync.dma_start(
                            oh_sb[:, 0:totg * 64],
                            oh_d.ap()[:, g0 * 64:(g0 + totg) * 64])
                        msg = []
                        for qi in range(NQ):
                            gq0 = int(gs[g, qi])
                            gcq = int(gc[g, qi])
                            mt = mpool.tile([128, MAXGC * ELMAX], bf16,
                                            name=f"m{rep}_{l}_{g}_{qi}",
                                            tag=f"msg{qi}", bufs=2)
                            mv = mt[:, 0:gcq * elem].rearrange(
                                "p (c e) -> p c e", e=elem)
                            _dma_gather_small(
                                nc.gpsimd, mv,
                                ytab_cur[qi * QROWS:(qi + 1) * QROWS, 0:elem],
                                idx_sb[:, (gq0 - g0) * 8:(gq0 - g0 + gcq) * 8],
                                gcq * 128, elem, PITCH, queue_num=qi)
                            nc.vector.tensor_tensor(
                                out=mv[:, :, 0:cg], in0=mv[:, :, 0:cg],
                                in1=wts_sb[:, gq0:gq0 + gcq].unsqueeze(2)
                                    .broadcast_to((128, gcq, cg)),
                                op=OP.mult)
                            msg.append(mv)
                        for wi in range(g * WPG, (g + 1) * WPG):
                            zps = zpool.tile([W, W], f32,
                                             name=f"z{rep}_{l}_{wi}", tag="zps",
                                             bufs=2)
                            first = True
                            for qi in range(NQ):
                                lc0 = int(cid0[wi, qi] - gs[g, qi])
                                for k in range(int(NCH[wi, qi])):
                                    ohc = int(cid0[wi, qi] + k - g0) * 64
                                    nc.tensor.matmul(
                                        out=zps[:, 0:cg],
                                        lhsT=oh_sb[:, ohc:ohc + 64],
                                        rhs=msg[qi][:, lc0 + k, 0:cg],
                                        start=first, stop=False)
                                    first = False
                            nc.tensor.matmul(
                                out=zps[:, 0:cg],
                                lhsT=xT_cur[:, wi * W:(wi + 1) * W],
                                rhs=wroot_sb[l][:],
                                start=first, stop=True)
                            zsb = ypool.tile([W, W], bf16,
                                             name=f"zs{rep}_{l}_{wi}", tag="zsb",
                                             bufs=3)
                            nc.scalar.activation(out=zsb[:, 0:cg],
                                                 in_=zps[:, 0:cg], func=AF.Copy)
                            pst = tpool.tile([W, W], bf16,
                                             name=f"pt{rep}_{l}_{wi}", tag="pst",
                                             bufs=2)
                            nc.tensor.transpose(pst[0:cg, :], zsb[:, 0:cg],
                                                ident_sb[:])
                            if l < L - 1:
                                nc.scalar.activation(
                                    out=xT_next[:, wi * W:(wi + 1) * W],
                                    in_=pst[0:cg, :], func=AF.Relu,
                                    bias=brel_sb[l][:])
                            else:
                                zrt = ypool.tile([2, W], bf16,
                                                 name=f"zr{rep}_{wi}", tag="zrt",
                                                 bufs=3)
                                nc.scalar.activation(
                                    out=zrt[:], in_=pst[0:2, :], func=AF.Relu,
                                    bias=brel_sb[l][:])
                                dps = zpool.tile([2, W], f32,
                                                 name=f"dp{rep}_{wi}", tag="dps",
                                                 bufs=1)
                                nc.tensor.matmul(out=dps[:], lhsT=dv_sb[:],
                                                 rhs=zrt[:], start=True,
                                                 stop=True)
                                nc.scalar.activation(
                                    out=pout[:, wi * W:(wi + 1) * W],
                                    in_=dps[:], func=AF.Sigmoid)
                        if l < L - 1:
                            build_table(xT_next, l + 1, g * GIPG,
                                        (g + 1) * GIPG, ybsb)
                    if l < L - 1:
                        flush_table(l + 1, ybsb)
                        xT_cur = xT_next
                nc.sync.dma_start(out_d.ap()[:], pout[:, 0:NPC])

            for rep in range(reps):
                emit_body(rep)
    return


def make_host_inputs(inputs):
    x = np.asarray(inputs["x"], np.float32)
    plan, percore = preprocess(inputs["edge_index"], inputs["edge_weight"])
    common = {
        "ident": np.eye(64, dtype=np.float32).astype(bf),
        "dv": np.array([[1.0, -1.0], [-1.0, 1.0]], np.float32).astype(bf),
    }
    for l in range(L):
        common[f"wrel{l}"] = np.asarray(inputs[f"w_rel{l}"], np.float32).astype(bf)
        common[f"wroot{l}"] = np.asarray(inputs[f"w_root{l}"], np.float32).astype(bf)
        common[f"brel{l}"] = np.asarray(
            inputs[f"b_rel{l}"], np.float32).reshape(-1, 1)
    in_maps = []
    for c in range(N_CORES):
        m = dict(common)
        m.update(percore[c])
        xT0 = np.zeros((DIMS[0], NPCP), np.float32)
        xT0[:, 0:NPC] = x[c * NPC:(c + 1) * NPC, :].T
        m["xT0"] = xT0.astype(bf)
        in_maps.append(m)
    return in_maps, plan


def _install_loud_hook():
    import traceback
    from concourse import bass2jax
    bass2jax.install_neuronx_cc_hook()
    try:
        import libneuronxla
    except ImportError:
        return
    hook = libneuronxla.neuronx_cc
    def loud(*a, **k):
        try:
            return hook(*a, **k)
        except BaseException:
            traceback.print_exc()
            raise
    libneuronxla.neuronx_cc = loud
    bass2jax.install_neuronx_cc_hook = lambda: None


def run_gnn(inputs, trace=False, reps=1):
    import concourse.bacc as bacc
    from concourse.bass_utils import run_bass_kernel_spmd
    _install_loud_hook()
    in_maps, plan = make_host_inputs(inputs)
    nc = bacc.Bacc("TRN2", target_bir_lowering=False, debug=False,
                   num_devices=N_CORES, num_swdge_queues=NQ)
    build_gnn(nc, plan, reps=reps)
    nc.compile()
    res = run_bass_kernel_spmd(nc, in_maps, core_ids=list(range(N_CORES)),
                               trace=trace)
    out = np.zeros((N_NODES, 2), np.float32)
    for c in range(N_CORES):
        o = res.results[c]["out"]          # [2, NPC]
        out[c * NPC:(c + 1) * NPC, 0] = o[0]
        out[c * NPC:(c + 1) * NPC, 1] = o[1]
    return out, res


def kernel(**inputs):
    out, _ = run_gnn(inputs)
    return out


def kernel_traced(**inputs):
    return run_gnn(inputs, trace=True)
